# revision 50
# baseline (speedup 1.0000x reference)
"""2-layer GCN (GCNConv -> ReLU -> GCNConv -> edge dot products) on 8 TRN2
NeuronCores via Bass/Tile.

Math: with A' = A + I (self loops), deg = in-degree of A', dinv = deg^-1/2:
    h  = relu(dinv_d * sum_{e->d} [dinv_s * (x_s @ W1)] + b1)
    z  = dinv_d * sum_{e->d} [dinv_s * (h_s @ W2)] + b2
    out[k] = dot(z[src_k], z[dst_k])  over edge_label_index
The symmetric norm factors out of the edge sum: we scale table rows by dinv
before the gather and scale the aggregated result by dinv after.

Sharding: nodes are range-sharded over 8 cores (12500 each, padded to
NP=12544); edges are partitioned by destination core, sorted by (dst tile,
src chunk, src row).  Gather tables are split into K=2 row-chunks, each
AllGathered separately so the collective for chunk 0 (produced by the first
half of the tiles) overlaps the producer's second half and the consumer's
chunk-0 gathers overlap the chunk-1 collective.  Each core:
  stage0: xw1' = dinv * (x_shard @ W1)        -> 2 chunked AllGathers (bf16)
  L1:     per 128-dst tile, gather xw1'[src] rows (dma_gather), build the
          one-hot P for all blocks of a tile with ONE broadcast is_equal
          (iota[128]x1 vs dstloc[1]x128 -> [128,nb,128]), accumulate
          P^T @ G in PSUM, post-scale + bias + relu -> h; hw2' = dinv *
          (h @ W2) -> 2 chunked AllGathers
  L2:     same aggregation over hw2' -> z shard, stored as bf16 [z_i ; z_i]
          duplicated rows (halves the collective wire vs f32 while keeping
          the 256B gather-element minimum) -> 2 chunked AllGathers
  final:  gather z[src], z[dst] for its 25k label pairs (grouped by chunk
          pair so group (0,0) only waits on the first z collective),
          multiply + reduce over the first 64 lanes.

dma_gather constraints shape the layout: int16 indices (signed offsets from
row 32768 of a 50176-row chunk table; negative offsets address below the
base and work on HW), <=1024 indices per single-packet call (multi-packet
2048/4096 calls run correctly but are slower end to end; 2048 with
single_packet=True wedges the device), elem_size a multiple of 256B, and
trailing-negative indices terminate the stream -- so partition 127 of every
128-slot block is a pad (idx 0), keeping each call's last slot non-negative.
Host-side work is index manipulation only (bucketing/sorting/padding) plus
the degree histogram that falls out of the dst partition; all floating-point
math runs on device.
"""
import os
import sys

sys.path.insert(0, "/opt/trn_rl_repo")

import numpy as np
import ml_dtypes

# debug bisection: 0=stage0+AG1, 1=+L1+AG2, 2=+L2+AG3, 3=full (default)
PHASE = int(os.environ.get("GCN_PHASE", "3"))
# timing: emit the whole body R times
REPEAT = int(os.environ.get("GCN_REPEAT", "1"))
# replace collectives with local DMA copies (single-core timeline analysis)
NOCC = bool(int(os.environ.get("GCN_NOCC", "0")))

import concourse.bacc as bacc
import concourse.bass as bass
import concourse.mybir as mybir
import concourse.tile as tile
from concourse.bass import broadcast_tensor_aps
from concourse.bass_utils import run_bass_kernel_spmd

F32 = mybir.dt.float32
BF16 = mybir.dt.bfloat16
I16 = mybir.dt.int16

N = 100000
NCORES = 8
NS = N // NCORES            # 12500 nodes per core
T = (NS + 127) // 128       # 98 dst tiles per core
NP = T * 128                # padded shard nodes (12544)
C_IN = 256
HID = 128
OUT = 64
NW = 2                      # src chunks (each its own AllGathered table)
HNP = NP // NW              # 6272 rows per chunk shard
TH = T // NW                # 49 tiles per chunk
TABR = NCORES * HNP         # 50176 rows per chunk table
RBASE = 32768               # int16 offsets are relative to this table row
BT = 7                      # dst tiles per gather batch
NBATCH = T // BT            # 14
E_LBL = 200000
LS = E_LBL // NCORES        # 25000 label pairs per core
EPB = 127                   # edges per 128-slot block (slot 127 = pad)
CALL_BLOCKS = 8             # max blocks per edge dma_gather call: 1024 idx
                            # in one packet/engine.  Multi-packet 2048/4096
                            # calls run correctly but measured +200/+400us
                            # end-to-end (coarser completion granularity).
LCALL_BLOCKS = 8            # max blocks per label dma_gather call
NQ = 4                      # SWDGE queues used for gathers (ucode max)

# exported for test harness introspection
LAST_RESULTS = None

_PROGRAM_CACHE = {}


# --------------------------------------------------------------- static layout

def _layout(cfg):
    """All static offsets derived from cfg = (nbw, lg).

    nbw: per-tile (nb_chunk0, nb_chunk1) block budgets.  lg: 4 label-group
    budgets.
    """
    nbw, lg = cfg
    tb_off = []                 # dstloc/matmul column base per tile
    s = 0
    for t in range(T):
        tb_off.append(s)
        s += nbw[t][0] + nbw[t][1]
    TB = s                      # total blocks per layer pass

    batches = []
    ecol = 0                    # running eidx int16 column offset
    for b in range(NBATCH):
        tiles = list(range(b * BT, (b + 1) * BT))
        reg = [sum(nbw[t][w] for t in tiles) for w in range(NW)]
        gcol = {}               # (tile, w) -> G column base for this batch
        for w in range(NW):
            base = 0 if w == 0 else reg[0]
            for t in tiles:
                gcol[(t, w)] = base
                base += nbw[t][w]
        calls = []              # (w, g_col, nblocks, ecol_off)
        for w in range(NW):
            off = 0
            while off < reg[w]:
                nb = min(reg[w] - off, CALL_BLOCKS)
                calls.append((w, (0 if w == 0 else reg[0]) + off, nb, ecol))
                ecol += nb * 8
                off += nb
        batches.append({"tiles": tiles, "reg": reg, "gcol": gcol,
                        "calls": calls})
    ecols = ecol

    lgoff = [0]
    for v in lg:
        lgoff.append(lgoff[-1] + v)
    lblk = lgoff[-1]
    pieces = []                 # (w1, w2, block_off, nblocks)
    for g in range(NW * NW):
        nb = lg[g]
        off = lgoff[g]
        while nb > 0:
            take = min(nb, LCALL_BLOCKS)
            pieces.append((g // NW, g % NW, off, take))
            off += take
            nb -= take
    return {"tb_off": tb_off, "TB": TB, "batches": batches, "ecols": ecols,
            "lgoff": lgoff, "lblk": lblk, "pieces": pieces}


# ----------------------------------------------------------------- host prep

def _pack_idx(flat):
    """dma_gather index layout: arr[j, i] = flat[i*16 + j], tiled to 128."""
    arr = np.asarray(flat, dtype=np.int16).reshape(-1, 16).T
    return np.tile(arr, (8, 1))


def _fill_blocks(flat, base_slot, values):
    """Place `values` into 128-slot blocks at flat[base_slot:], 127 per block
    (slot 127 reserved as pad)."""
    i = np.arange(len(values))
    pos = base_slot + (i // EPB) * 128 + (i % EPB)
    flat[pos] = values


def _node_row(v):
    """Padded chunk-table mapping: node v -> (chunk, row in chunk table)."""
    c = v // NS
    l = v % NS
    w = (l >= HNP).astype(np.int64)
    return w, c * HNP + (l - w * HNP)


def _prep(x, edge_index, edge_label_index, W1, b1, W2, b2):
    src = np.asarray(edge_index[0], dtype=np.int64)
    dst = np.asarray(edge_index[1], dtype=np.int64)
    ar = np.arange(N, dtype=np.int64)
    src_all = np.concatenate([src, ar])
    dst_all = np.concatenate([dst, ar])

    # degree histogram falls out of the dst partitioning
    deg = np.bincount(dst_all, minlength=N).astype(np.float32)

    core_of = dst_all // NS

    per_core = []
    cnts = np.zeros((NCORES, T, NW), np.int64)
    for c in range(NCORES):
        m = core_of == c
        s = src_all[m]
        dl = dst_all[m] - c * NS
        tl = dl >> 7
        loc = (dl & 127).astype(np.float32)
        w, row = _node_row(s)
        order = np.lexsort((row, w, tl))  # by tile, chunk, then row (locality)
        row, tl, loc, w = row[order], tl[order], loc[order], w[order]
        cnt = np.bincount(tl * NW + w, minlength=T * NW).reshape(T, NW)
        cnts[c] = cnt
        per_core.append((row, loc, cnt))

    nbw = tuple(
        tuple(int(v) for v in
              np.ceil(cnts[:, t, :].max(axis=0) / EPB).astype(np.int64))
        for t in range(T))

    # label pairs: shard by index, double-bucket by (src chunk, dst chunk)
    lsrc = np.asarray(edge_label_index[0], dtype=np.int64)
    ldst = np.asarray(edge_label_index[1], dtype=np.int64)
    lab_core = []
    lcnts = np.zeros((NCORES, NW * NW), np.int64)
    for c in range(NCORES):
        ls_ = lsrc[c * LS:(c + 1) * LS]
        ld_ = ldst[c * LS:(c + 1) * LS]
        w1_, r1_ = _node_row(ls_)
        w2_, r2_ = _node_row(ld_)
        order = np.lexsort((w2_, w1_))
        g = (w1_ * NW + w2_)[order]
        lcnts[c] = np.bincount(g, minlength=NW * NW)
        lab_core.append((r1_[order], r2_[order], order))
    lg = tuple(int(v) for v in
               np.ceil(lcnts.max(axis=0) / EPB).astype(np.int64))

    cfg = (nbw, lg)
    lay = _layout(cfg)

    iota = np.broadcast_to(np.arange(128, dtype=np.float32),
                           (128, 128)).astype(ml_dtypes.bfloat16)
    ident = np.eye(128, dtype=np.float32).astype(ml_dtypes.bfloat16)
    w1m = np.asarray(W1, np.float32).astype(ml_dtypes.bfloat16)      # [256,128]
    w2p = np.zeros((HID, 128), np.float32)
    w2p[:, :OUT] = np.asarray(W2, np.float32)
    w2p = w2p.astype(ml_dtypes.bfloat16)
    b1r = np.broadcast_to(np.asarray(b1, np.float32), (128, HID)).copy()
    b2r = np.broadcast_to(np.asarray(b2, np.float32), (128, OUT)).copy()

    xf = np.asarray(x, np.float32)

    in_maps = []
    slot2orig = []
    for c in range(NCORES):
        row, loc, cnt = per_core[c]
        gstart = np.concatenate([[0], np.cumsum(cnt.reshape(-1))]).astype(np.int64)

        eflat = np.zeros(lay["ecols"] * 16, np.int16)
        dlflat = np.full(lay["TB"] * 128, 255.0, np.float32)
        for b in range(NBATCH):
            bi = lay["batches"][b]
            # eidx slot base of this batch's stream = 16 * ecol of first call
            sbase = bi["calls"][0][3] * 16
            for w in range(NW):
                for t in bi["tiles"]:
                    n_e = int(cnt[t, w])
                    if not n_e:
                        continue
                    gi = t * NW + w
                    vals = (row[gstart[gi]:gstart[gi] + n_e]
                            - RBASE).astype(np.int16)
                    _fill_blocks(eflat, sbase + bi["gcol"][(t, w)] * 128, vals)
                    dcol = lay["tb_off"][t] + (nbw[t][0] if w else 0)
                    _fill_blocks(dlflat, dcol * 128,
                                 loc[gstart[gi]:gstart[gi] + n_e])
        eidx = _pack_idx(eflat)
        dstloc = dlflat.reshape(lay["TB"], 128).T.astype(ml_dtypes.bfloat16)

        # label indices
        r1_, r2_, order = lab_core[c]
        lcnt = lcnts[c]
        lblk = lay["lblk"]
        lsflat = np.zeros(lblk * 128, np.int16)
        ldflat = np.zeros(lblk * 128, np.int16)
        s2o = np.full(lblk * 128, -1, np.int64)
        pos = 0
        for g in range(NW * NW):
            n_p = int(lcnt[g])
            base = lay["lgoff"][g] * 128
            if n_p:
                _fill_blocks(lsflat, base,
                             (r1_[pos:pos + n_p] - RBASE).astype(np.int16))
                _fill_blocks(ldflat, base,
                             (r2_[pos:pos + n_p] - RBASE).astype(np.int16))
                _fill_blocks(s2o, base, order[pos:pos + n_p])
            pos += n_p
        slot2orig.append(s2o)

        xs = xf[c * NS:(c + 1) * NS]
        xT = np.zeros((C_IN, NP), np.float32)
        xT[:, :NS] = xs.T
        degc = np.ones(NP, np.float32)
        degc[:NS] = deg[c * NS:(c + 1) * NS]

        in_maps.append({
            "xT": xT.astype(ml_dtypes.bfloat16),
            "w1": w1m, "w2p": w2p, "b1r": b1r, "b2r": b2r,
            "iota": iota, "ident": ident,
            "deg": degc.reshape(T, 128).T.copy(),
            "dstloc": dstloc,
            "eidx": eidx,
            "lsidx": _pack_idx(lsflat),
            "ldidx": _pack_idx(ldflat),
        })
    return cfg, in_maps, slot2orig


# ------------------------------------------------------------- device program

def _build(cfg):
    nbw, lg = cfg
    lay = _layout(cfg)
    TB = lay["TB"]
    ecols = lay["ecols"]
    lblk = lay["lblk"]
    lcols = lblk * 8
    gwv = [max(bi["reg"][w] for bi in lay["batches"]) for w in range(NW)]
    nbmax = max(nbw[t][0] + nbw[t][1] for t in range(T))

    nc = bacc.Bacc("TRN2", target_bir_lowering=False, debug=False,
                   num_devices=1 if NOCC else NCORES, num_swdge_queues=NQ)

    xT_d = nc.dram_tensor("xT", [C_IN, NP], BF16, kind="ExternalInput")
    w1_d = nc.dram_tensor("w1", [C_IN, HID], BF16, kind="ExternalInput")
    w2p_d = nc.dram_tensor("w2p", [HID, 128], BF16, kind="ExternalInput")
    b1r_d = nc.dram_tensor("b1r", [128, HID], F32, kind="ExternalInput")
    b2r_d = nc.dram_tensor("b2r", [128, OUT], F32, kind="ExternalInput")
    iota_d = nc.dram_tensor("iota", [128, 128], BF16, kind="ExternalInput")
    ident_d = nc.dram_tensor("ident", [128, 128], BF16, kind="ExternalInput")
    deg_d = nc.dram_tensor("deg", [128, T], F32, kind="ExternalInput")
    dstloc_d = nc.dram_tensor("dstloc", [128, TB], BF16, kind="ExternalInput")
    eidx_d = nc.dram_tensor("eidx", [128, ecols], I16, kind="ExternalInput")
    lsidx_d = nc.dram_tensor("lsidx", [128, lcols], I16, kind="ExternalInput")
    ldidx_d = nc.dram_tensor("ldidx", [128, lcols], I16, kind="ExternalInput")
    out_d = nc.dram_tensor("out_lbl", [128, lblk], F32, kind="ExternalOutput")
    if PHASE == 0:
        dbg_d = nc.dram_tensor("dbg", [NW * TABR, HID], BF16,
                               kind="ExternalOutput")
    elif PHASE == 1:
        dbg_d = nc.dram_tensor("dbg", [NW * TABR, 128], BF16,
                               kind="ExternalOutput")
    elif PHASE == 2:
        dbg_d = nc.dram_tensor("dbg", [NW * TABR, 2 * OUT], BF16,
                               kind="ExternalOutput")

    dum_in = nc.dram_tensor("dum_in", [128, 16], BF16)
    dum_out = nc.dram_tensor("dum_out", [128 * NCORES, 16], BF16,
                             addr_space="Shared")
    cc0_in = [nc.dram_tensor(f"cc0_in{k}", [HNP, HID], BF16)
              for k in range(NW)]
    cc0_out = [nc.dram_tensor(f"cc0_out{k}", [TABR, HID], BF16,
                              addr_space="Shared") for k in range(NW)]
    cc1_in = [nc.dram_tensor(f"cc1_in{k}", [HNP, 128], BF16)
              for k in range(NW)]
    cc1_out = [nc.dram_tensor(f"cc1_out{k}", [TABR, 128], BF16,
                              addr_space="Shared") for k in range(NW)]
    # z table rows are [z_i ; z_i] in bf16: the duplicate halves the
    # collective wire vs f32 while keeping the 256B gather-element minimum.
    cc2_in = [nc.dram_tensor(f"cc2_in{k}", [HNP, 2 * OUT], BF16)
              for k in range(NW)]
    cc2_out = [nc.dram_tensor(f"cc2_out{k}", [TABR, 2 * OUT], BF16,
                              addr_space="Shared") for k in range(NW)]

    rg = [list(range(NCORES))]
    mult = mybir.AluOpType.mult
    add = mybir.AluOpType.add
    iseq = mybir.AluOpType.is_equal
    Relu = mybir.ActivationFunctionType.Relu

    def ag(cin, cout, after=()):
        if NOCC:
            return nc.sync.dma_start(cout[0:HNP, :], cin[:])
        cc = nc.gpsimd.collective_compute(
            "AllGather", mybir.AluOpType.bypass, replica_groups=rg,
            ins=[cin[:]], outs=[cout[:]])
        # Explicit ordering edges: the Tile scheduler otherwise places the
        # collective before these gathers on the GpSimd FIFO (its collective
        # cost model overestimates the predecessor AG, making the gathers
        # look not-ready), and the blocking collective then starves the
        # SDMA engines for its whole duration.
        for gi in after:
            bass._add_dep_helper(cc.ins, gi.ins, sync=False,
                                 reason="order collective after prefetch")
        return cc

    with tile.TileContext(nc) as tc:
        with tc.tile_pool(name="const", bufs=1) as cpool, \
             tc.tile_pool(name="work", bufs=2) as wpool, \
             tc.tile_pool(name="gbuf", bufs=3) as gpool, \
             tc.tile_pool(name="pbuf", bufs=3) as ppool, \
             tc.tile_pool(name="psum", bufs=2, space="PSUM") as pspool:

            # ---- index tables first: the first gathers wait on these
            eidx_sb = cpool.tile([128, ecols], I16)
            nc.sync.dma_start(eidx_sb[:], eidx_d[:])
            lsidx_sb = cpool.tile([128, lcols], I16)
            nc.sync.dma_start(lsidx_sb[:], lsidx_d[:])
            ldidx_sb = cpool.tile([128, lcols], I16)
            nc.sync.dma_start(ldidx_sb[:], ldidx_d[:])

            # ---- constants
            iota3 = cpool.tile([128, 1, 128], BF16)
            nc.sync.dma_start(iota3[:, 0, :], iota_d[:])
            ident_sb = cpool.tile([128, 128], BF16)
            nc.sync.dma_start(ident_sb[:], ident_d[:])
            b1r_sb = cpool.tile([128, HID], F32)
            nc.sync.dma_start(b1r_sb[:], b1r_d[:])
            b2r_sb = cpool.tile([128, OUT], F32)
            nc.sync.dma_start(b2r_sb[:], b2r_d[:])
            w1_sb = cpool.tile([128, 2, HID], BF16)
            nc.sync.dma_start(w1_sb[:, 0, :], w1_d[0:128, :])
            nc.sync.dma_start(w1_sb[:, 1, :], w1_d[128:256, :])
            w2p_sb = cpool.tile([128, 128], BF16)
            nc.sync.dma_start(w2p_sb[:], w2p_d[:])
            dstloc_sb = cpool.tile([128, TB, 1], BF16)
            nc.sync.dma_start(dstloc_sb[:, :, 0], dstloc_d[:])

            deg_sb = cpool.tile([128, T], F32)
            nc.sync.dma_start(deg_sb[:], deg_d[:])
            rec_sb = cpool.tile([128, T], F32)
            nc.vector.reciprocal(rec_sb[:], deg_sb[:])
            dinv = cpool.tile([128, T], F32)
            nc.scalar.sqrt(dinv[:], rec_sb[:])

            qctr = [0]

            # per-chunk rearranged store views: [128 p, TH t, ch]
            cc0_r = [t_[:].rearrange("(t p) h -> p t h", p=128)
                     for t_ in cc0_in]
            cc1_r = [t_[:].rearrange("(t p) h -> p t h", p=128)
                     for t_ in cc1_in]
            cc2_r = [t_[:].rearrange("(t p) h -> p t h", p=128)
                     for t_ in cc2_in]

            def emit_body():
                # Tiny warm-up collective: absorbs the cross-core rendezvous
                # and CC warmup while the input DMAs stream in, so the first
                # real AllGather starts at wire speed.
                ag(dum_in, dum_out)

                # ---- stage 0: xw1' = dinv * (x @ W1), bf16 chunked table
                for b in range(NBATCH):
                    xb = wpool.tile([128, 2, BT * 128], BF16, tag="xb")
                    nc.sync.dma_start(
                        xb[:, 0, :], xT_d[0:128, b * BT * 128:
                                          (b + 1) * BT * 128])
                    nc.sync.dma_start(
                        xb[:, 1, :], xT_d[128:256, b * BT * 128:
                                          (b + 1) * BT * 128])
                    tbb = wpool.tile([128, BT, HID], BF16, tag="tb0")
                    for i, t in enumerate(range(b * BT, (b + 1) * BT)):
                        ps = pspool.tile([128, HID], F32, tag="ps0")
                        nc.tensor.matmul(ps[:],
                                         lhsT=xb[:, 0,
                                                 i * 128:(i + 1) * 128],
                                         rhs=w1_sb[:, 0, :],
                                         start=True, stop=False)
                        nc.tensor.matmul(ps[:],
                                         lhsT=xb[:, 1,
                                                 i * 128:(i + 1) * 128],
                                         rhs=w1_sb[:, 1, :],
                                         start=False, stop=True)
                        nc.vector.tensor_scalar(out=tbb[:, i, :], in0=ps[:],
                                                scalar1=dinv[:, t:t + 1],
                                                scalar2=None, op0=mult)
                    k, tb0 = divmod(b, NBATCH // NW)
                    nc.sync.dma_start(
                        cc0_r[k][:, tb0 * BT:(tb0 + 1) * BT, :], tbb[:])
                    if b == NBATCH // NW - 1:
                        ag(cc0_in[0], cc0_out[0])

                def agg_layer(tables, n_ch, post_fn, width, store_fn,
                              agb=None):
                    # One G tile per (batch, window): finer slot release
                    # than a combined tile, so several batches' chunk-0
                    # gathers can be in flight while a chunk-1 collective
                    # blocks the GpSimd FIFO.
                    def alloc_g(w):
                        gwhite = gpool.tile([128, gwv[w], n_ch], BF16,
                                            tag=f"G{w}")
                        return gwhite

                    def emit_calls(gs, bi, which):
                        insts = []
                        for (w, g_col, nb, ecol) in bi["calls"]:
                            if w not in which:
                                continue
                            lcol = g_col - (0 if w == 0 else bi["reg"][0])
                            nidx = nb * 128
                            insts.append(nc.gpsimd.dma_gather(
                                gs[w][:, lcol:lcol + nb, :],
                                tables[w][RBASE:, :],
                                eidx_sb[:, ecol:ecol + nidx // 16],
                                nidx, nidx, n_ch,
                                single_packet=nidx <= 1024,
                                queue_num=qctr[0] % NQ))
                            qctr[0] += 1
                        return insts

                    gtiles = {}
                    if agb is not None:
                        pre = []
                        for bn in range(3):
                            gs = {0: alloc_g(0)}
                            gtiles[bn] = gs
                            pre += emit_calls(gs, lay["batches"][bn], (0,))
                        agb(pre)
                    for bn, bi in enumerate(lay["batches"]):
                        if bn in gtiles:
                            gs = gtiles.pop(bn)
                            gs[1] = alloc_g(1)
                            emit_calls(gs, bi, (1,))
                        else:
                            gs = {0: alloc_g(0), 1: alloc_g(1)}
                            emit_calls(gs, bi, (0, 1))
                        for i, t in enumerate(bi["tiles"]):
                            nbt_t = nbw[t][0] + nbw[t][1]
                            dcol0 = lay["tb_off"][t]
                            # one broadcast is_equal builds the one-hot P for
                            # every block of this tile: [128e, nbt, 128dst]
                            p3 = ppool.tile([128, nbmax, 128], BF16, tag="P")
                            b0, b1 = broadcast_tensor_aps(
                                iota3[:, :, :],
                                dstloc_sb[:, dcol0:dcol0 + nbt_t, :])
                            nc.vector.tensor_tensor(
                                out=p3[:, 0:nbt_t, :], in0=b0, in1=b1,
                                op=iseq)
                            ps = pspool.tile([128, width], F32, tag="agg")
                            k = 0
                            for w in range(NW):
                                for j in range(nbw[t][w]):
                                    col = (bi["gcol"][(t, w)] + j
                                           - (0 if w == 0
                                              else bi["reg"][0]))
                                    nc.tensor.matmul(
                                        ps[:], lhsT=p3[:, k, :],
                                        rhs=gs[w][:, col, :width],
                                        start=(k == 0),
                                        stop=(k == nbt_t - 1))
                                    k += 1
                            post_fn(t, i, ps)
                        store_fn(bn)

                # ---- L1 post: h = relu(.); hw2' = dinv * (h @ W2)
                h2b_box = [None]

                def post_l1(t, i, ps):
                    if i == 0:
                        h2b = wpool.tile([128, BT, 128], BF16, tag="h2b")
                        h2b_box[0] = h2b
                    tmp = wpool.tile([128, HID], F32, tag="tmp1")
                    nc.vector.scalar_tensor_tensor(
                        out=tmp[:], in0=ps[:], scalar=dinv[:, t:t + 1],
                        in1=b1r_sb[:], op0=mult, op1=add)
                    hsb = wpool.tile([128, HID], BF16, tag="hsb")
                    nc.scalar.activation(hsb[:], tmp[:], Relu)
                    psT = pspool.tile([128, 128], BF16, tag="psT")
                    nc.tensor.transpose(psT[:], hsb[:], ident_sb[:])
                    hT = wpool.tile([128, 128], BF16, tag="hT")
                    nc.vector.tensor_copy(hT[:], psT[:])
                    ps2 = pspool.tile([128, 128], F32, tag="hw2")
                    nc.tensor.matmul(ps2[:], lhsT=hT[:], rhs=w2p_sb[:],
                                     start=True, stop=True)
                    nc.vector.tensor_scalar(out=h2b_box[0][:, i, :],
                                            in0=ps2[:],
                                            scalar1=dinv[:, t:t + 1],
                                            scalar2=None, op0=mult)

                def store_l1(b):
                    k, tb0 = divmod(b, NBATCH // NW)
                    nc.sync.dma_start(
                        cc1_r[k][:, tb0 * BT:(tb0 + 1) * BT, :],
                        h2b_box[0][:])
                    if b == NBATCH // NW - 1:
                        ag(cc1_in[0], cc1_out[0])

                zb_box = [None]

                def post_l2(t, i, ps):
                    if i == 0:
                        zb = wpool.tile([128, BT, 2 * OUT], BF16, tag="zb")
                        zb_box[0] = zb
                    for h in range(2):
                        nc.vector.scalar_tensor_tensor(
                            out=zb_box[0][:, i, h * OUT:(h + 1) * OUT],
                            in0=ps[:], scalar=dinv[:, t:t + 1],
                            in1=b2r_sb[:], op0=mult, op1=add)

                def store_l2(b):
                    k, tb0 = divmod(b, NBATCH // NW)
                    nc.sync.dma_start(
                        cc2_r[k][:, tb0 * BT:(tb0 + 1) * BT, :],
                        zb_box[0][:])
                    if b == NBATCH // NW - 1:
                        ag(cc2_in[0], cc2_out[0])

                if PHASE == 0:
                    ag(cc0_in[1], cc0_out[1])
                    for k in range(NW):
                        nc.sync.dma_start(
                            dbg_d[k * TABR:(k + 1) * TABR, :],
                            cc0_out[k][:])

                if PHASE >= 1:
                    agg_layer(cc0_out, HID, post_l1, HID, store_l1,
                              agb=lambda pre: ag(cc0_in[1], cc0_out[1],
                                                 after=pre))
                if PHASE == 1:
                    ag(cc1_in[1], cc1_out[1])
                    for k in range(NW):
                        nc.sync.dma_start(
                            dbg_d[k * TABR:(k + 1) * TABR, :],
                            cc1_out[k][:])

                if PHASE >= 2:
                    agg_layer(cc1_out, 128, post_l2, OUT, store_l2,
                              agb=lambda pre: ag(cc1_in[1], cc1_out[1],
                                                 after=pre))
                if PHASE == 2:
                    ag(cc2_in[1], cc2_out[1])
                    for k in range(NW):
                        nc.sync.dma_start(
                            dbg_d[k * TABR:(k + 1) * TABR, :],
                            cc2_out[k][:])

                if PHASE >= 3:
                    # ---- final: label-edge dot products.  Pieces are in
                    # chunk-group order; after the (0,0) group's gathers are
                    # queued, trigger the deferred chunk-1 z collective.
                    out_sb = cpool.tile([128, lblk], F32, tag="out_sb")
                    ag3b = [False]
                    g00 = []
                    for (w1_, w2_, po, nb) in lay["pieces"]:
                        if (w1_ or w2_) and not ag3b[0]:
                            ag(cc2_in[1], cc2_out[1], after=g00)
                            ag3b[0] = True
                        nidx = nb * 128
                        zs = wpool.tile([128, LCALL_BLOCKS, 2 * OUT], BF16,
                                        tag="zs")
                        g00.append(nc.gpsimd.dma_gather(
                            zs[:, 0:nb, :], cc2_out[w1_][RBASE:, :],
                            lsidx_sb[:, po * 8:po * 8 + nidx // 16],
                            nidx, nidx, 2 * OUT,
                            single_packet=nidx <= 1024,
                            queue_num=qctr[0] % NQ))
                        qctr[0] += 1
                        zd = wpool.tile([128, LCALL_BLOCKS, 2 * OUT], BF16,
                                        tag="zd")
                        g00.append(nc.gpsimd.dma_gather(
                            zd[:, 0:nb, :], cc2_out[w2_][RBASE:, :],
                            ldidx_sb[:, po * 8:po * 8 + nidx // 16],
                            nidx, nidx, 2 * OUT,
                            single_packet=nidx <= 1024,
                            queue_num=qctr[0] % NQ))
                        qctr[0] += 1
                        pr = wpool.tile([128, LCALL_BLOCKS, OUT], F32,
                                        tag="pr")
                        nc.vector.tensor_tensor(out=pr[:, 0:nb, :],
                                                in0=zs[:, 0:nb, 0:OUT],
                                                in1=zd[:, 0:nb, 0:OUT],
                                                op=mult)
                        nc.vector.tensor_reduce(out=out_sb[:, po:po + nb],
                                                in_=pr[:, 0:nb, :],
                                                axis=mybir.AxisListType.X,
                                                op=add)
                    nc.sync.dma_start(out_d[:], out_sb[:])

            for _rep in range(REPEAT):
                emit_body()

    nc.compile()
    return nc


def _get_program(cfg):
    if cfg not in _PROGRAM_CACHE:
        _PROGRAM_CACHE[cfg] = _build(cfg)
    return _PROGRAM_CACHE[cfg]


# ------------------------------------------------------------------ entrypoint

def kernel(x, edge_index, edge_label_index, W1, b1, W2, b2):
    global LAST_RESULTS
    cfg, in_maps, slot2orig = _prep(x, edge_index, edge_label_index,
                                    W1, b1, W2, b2)
    nc = _get_program(cfg)
    res = run_bass_kernel_spmd(nc, in_maps, core_ids=list(range(NCORES)))
    LAST_RESULTS = res
    out = np.empty(E_LBL, np.float32)
    for c in range(NCORES):
        vals = res.results[c]["out_lbl"].T.reshape(-1)   # slot-ordered
        s2o = slot2orig[c]
        valid = s2o >= 0
        out[c * LS + s2o[valid]] = vals[valid]
    return out
